# revision 16
# baseline (speedup 1.0000x reference)
"""LiteLinear (dense linear + per-token LoRA adapters) on 8 Trainium2 cores.

Sharding: data-parallel over tokens. Each core computes 1024 tokens:
  out = x @ W^T + bias + per-token LoRA delta.

Device kernel (per core), mixed bf16 / fp8 matmuls (fp32 PSUM accumulate):
  - Computes out^T [D_OUT x TOK]; host transposes back on assembly.
  - Stationary operand = weight sub-chunk [128d x 128o], moving = x^T
    [128d x 512tok]. x^T resident in SBUF (bf16, 2KB/partition/tile).
  - A_cat^T (the concatenated LoRA down-projections) is prepended to W^T
    as a 33rd output column tile, so h^T = A_cat @ x^T rides the same
    streamed matmul pipeline; its eviction is a DVE multiply with a
    host-built maskT (folds scalings + one-hot) producing hmask^T (bf16).
  - The combined [A|W]^T stream is re-laid-out on the host in quad-major
    form: one dma_start per 4 contraction chunks, 2KB contiguous lines.
  - The trailing 2*F8PAIRS k-chunks of every accumulation run as fp8-e4m3
    DoubleRow matmuls (2 k-chunks per instruction, native scale, same PSUM
    bank) -- 2x PE rate on that slice of the contraction for a measured
    rel-err of 1.56e-2 against the 2e-2 gate (error scales as
    sqrt(F8PAIRS) * 1.05e-2; 4 pairs would hit 90% of the gate).
  - Group 0 is width 4 (A + 3 W-tiles) so its matmul pace (8/k-chunk)
    stays behind the x^T DMA delivery pace; later groups are width 2 so
    only 4 PSUM banks are held per group and the next group's
    accumulation overlaps this group's delta/eviction.
  - 16 junk warmup matmuls run during the DMA preamble so the PE's HAM
    clock gate reaches 8/8 (2.4 GHz) before the first real matmul.
  - Per-token LoRA delta enters each out-tile as one extra accumulating
    matmul (lhsT=B_cat chunk bf16, rhs=hmask^T); bias folded into
    PSUM->SBUF eviction via per-partition tensor_scalar_add.
  - DMA rings: W quads on sync (HWDGE), x^T on scalar (HWDGE),
    consts + output tiles on gpsimd (SWDGE); the final o-tile's output
    rides sync to cut the kernel tail.
"""

import numpy as np

import sys

if "/opt/trn_rl_repo" not in sys.path:
    sys.path.insert(0, "/opt/trn_rl_repo")

import ml_dtypes

import concourse.bass as bass
import concourse.mybir as mybir
import concourse.tile as tile
from concourse import bacc
from concourse.bass_utils import run_bass_kernel_spmd

N_TOK = 8192
D_IN = 4096
D_OUT = 4096
N_ADAPTERS = 8
RANK = 16
AR = N_ADAPTERS * RANK  # 128
N_CORES = 8
TOK = N_TOK // N_CORES  # 1024 tokens per core

P = 128            # partitions
FREE = 512         # matmul moving free dim (== 1 PSUM bank of fp32)
KC = D_IN // P     # 32 contraction chunks
KQ = 4             # k-chunks per quad DMA
NQ = KC // KQ      # 8 quads
TH = TOK // FREE   # 2 token halves
NO = D_OUT // P + 1  # 33 o128-tiles incl. the A tile (index 0)
GROUPS = [4] + [2] * 14 + [1]  # o128-tiles per group (sum 33); g0 = [A,W0..W2]
NWARM = 16         # junk matmuls to lift the HAM clock gate during preamble
F8PAIRS = 4        # trailing k-chunk pairs done as fp8 DoubleRow (2 chunks/mm)
F8SCALE = 8.0      # power-of-2 split scale: W*s, x/s -- exact cancellation,
                   # rebalances both operands into e4m3 normal range
KC8 = KC - 2 * F8PAIRS  # k-chunks done in bf16 (28)

F32 = mybir.dt.float32
BF16 = mybir.dt.bfloat16
F8 = mybir.dt.float8e4

_CACHE = {}


def _build_nc():
    nc = bacc.Bacc(None, target_bir_lowering=False, debug=True)

    xT = nc.dram_tensor("xT", [D_IN, TOK], BF16, kind="ExternalInput")
    # quad-major [A|W]: [kq, p, (g kk cols_g)] with per-group contiguous blocks
    wTr = nc.dram_tensor("wTr", [NQ, P, KQ * NO * P], BF16,
                         kind="ExternalInput")
    bcat = nc.dram_tensor("bcat", [AR, D_OUT], BF16, kind="ExternalInput")
    maskT = nc.dram_tensor("maskT", [AR, TOK], F32, kind="ExternalInput")
    biasr = nc.dram_tensor("biasr", [P, D_OUT // P], F32, kind="ExternalInput")
    # trailing k-chunks in fp8, DoubleRow pair-major layouts
    x8 = nc.dram_tensor("x8", [F8PAIRS, P, 2, TOK], F8, kind="ExternalInput")
    w8r = nc.dram_tensor("w8r", [F8PAIRS, P, 2 * NO * P], F8,
                         kind="ExternalInput")
    outT = nc.dram_tensor("outT", [D_OUT, TOK], F32, kind="ExternalOutput")

    def w_src(kq, goff, coff, blk):
        """Slice of quad kq: per-(partition) rows, blk contiguous cols
        starting at goff+coff within the group-blocked column space."""
        return bass.AP(
            tensor=wTr[:].tensor,
            offset=kq * P * KQ * NO * P + goff + coff,
            ap=[[KQ * NO * P, P], [1, blk]],
        )

    def w8_src(q, goff8, blk):
        return bass.AP(
            tensor=w8r[:].tensor,
            offset=q * P * 2 * NO * P + goff8,
            ap=[[2 * NO * P, P], [1, blk]],
        )

    def x8_src(q):
        return bass.AP(
            tensor=x8[:].tensor,
            offset=q * P * 2 * TOK,
            ap=[[2 * TOK, P], [TOK, 2], [1, TOK]],
        )

    with tile.TileContext(nc) as tc:
        with (
            tc.tile_pool(name="xpool", bufs=1) as xpool,
            tc.tile_pool(name="const", bufs=1) as const,
            tc.tile_pool(name="wpool", bufs=3) as wpool,
            tc.tile_pool(name="opool", bufs=3) as opool,
            tc.tile_pool(name="psum", bufs=8, space="PSUM") as psum,
        ):
            hmask = const.tile([P, TOK], BF16, tag="hmask")
            biasr_sb = const.tile([P, D_OUT // P], F32, tag="biasr")
            maskT_sb = const.tile([P, TOK], F32, tag="maskT")
            bcat_sb = const.tile([P, D_OUT], BF16, tag="bcat")
            wjunk = const.tile([P, P], BF16, tag="wjunk")
            xjunk = const.tile([P, FREE], BF16, tag="xjunk")

            # PE warmup: junk matmuls with no DMA deps keep the PE busy
            # through the ~9us preamble so HAM unthrottles to 2.4 GHz.
            nc.vector.memset(wjunk[:], 0.0)
            nc.vector.memset(xjunk[:], 0.0)
            pwarm = [psum.tile([P, FREE], F32, tag="ps", name=f"pwarm{i}")
                     for i in range(2)]
            for i in range(NWARM):
                nc.tensor.matmul(pwarm[i % 2][:], wjunk[:], xjunk[:],
                                 start=True, stop=True)


            # consts ride the gpsimd ring (idle until output DMAs start)
            nc.gpsimd.dma_start(out=biasr_sb[:], in_=biasr[:, :])
            nc.gpsimd.dma_start(out=maskT_sb[:], in_=maskT[:, :])
            nc.gpsimd.dma_start(out=bcat_sb[:], in_=bcat[:, :])

            # x^T fully prefetched on the scalar ring (8 MB bf16)
            xt = []
            for k in range(KC):
                t = xpool.tile([P, TOK], BF16, tag=f"xt{k}", name=f"xt{k}")
                nc.scalar.dma_start(out=t[:], in_=xT[k * P:(k + 1) * P, :])
                xt.append(t)

            x8t = []
            for q in range(F8PAIRS):
                t = xpool.tile([P, 2, TOK], F8, tag=f"x8_{q}", name=f"x8_{q}")
                nc.gpsimd.dma_start(out=t[:], in_=x8_src(q))
                x8t.append(t)

            def moving(k, th):
                tsl = slice(th * FREE, (th + 1) * FREE)
                return xt[k][:, tsl]

            def base_loop(g, width, goff, goff8, pg):
                """28 bf16 k-chunks + 2 fp8-DR pairs for one o-group."""
                wt = None
                wt0 = [None] * KQ
                w8t = [None] * F8PAIRS
                for k in range(KC8):
                    if g == 0 and k < KQ:
                        # first quad split per-k for a faster first matmul
                        if k == 0:
                            for kk in range(KQ):
                                t = wpool.tile([P, width * P], BF16,
                                               tag="wt0", name=f"wt00_{kk}", bufs=4)
                                nc.sync.dma_start(
                                    out=t[:],
                                    in_=w_src(0, goff, kk * width * P,
                                              width * P))
                                wt0[kk] = t
                    elif k % KQ == 0:
                        wt = wpool.tile([P, KQ * width * P], BF16,
                                        tag=f"wt{width}", name=f"wt{g}_{k}")
                        nc.sync.dma_start(
                            out=wt[:],
                            in_=w_src(k // KQ, goff, 0, KQ * width * P))
                    if k == KC8 - KQ:
                        # fp8 stationaries ride sync behind the last quad
                        for q in range(F8PAIRS):
                            t8 = wpool.tile([P, 2, width * P], F8,
                                            tag=f"w8_{width}",
                                            name=f"w8_{g}_{q}",
                                            bufs=F8PAIRS + 1)
                            nc.sync.dma_start(
                                out=t8[:], in_=w8_src(q, goff8, 2 * width * P))
                            w8t[q] = t8
                    kk = k % KQ
                    for j in range(width):
                        if g == 0 and k < KQ:
                            lhsT = wt0[kk][:, j * P:(j + 1) * P]
                        else:
                            lhsT = wt[:, (kk * width + j) * P:
                                      (kk * width + j + 1) * P]
                        for th in range(TH):
                            nc.tensor.matmul(
                                pg[j * TH + th][:],
                                lhsT,
                                moving(k, th),
                                start=(k == 0),
                                stop=False,
                            )
                # trailing contraction: fp8 DoubleRow, 2 k-chunks per matmul
                for q in range(F8PAIRS):
                    for j in range(width):
                        lhsT8 = w8t[q][:, :, j * P:(j + 1) * P]
                        for th in range(TH):
                            tsl = slice(th * FREE, (th + 1) * FREE)
                            nc.tensor.matmul(
                                pg[j * TH + th][:],
                                lhsT8,
                                x8t[q][:, :, tsl],
                                start=False,
                                stop=(q == F8PAIRS - 1 and g == 0 and j == 0),
                                perf_mode=mybir.MatmulPerfMode.DoubleRow,
                            )

            ooff = 0  # in o128-tiles over the combined [A|W] column space
            last_g = len(GROUPS) - 1
            for g, width in enumerate(GROUPS):
                pg = [
                    psum.tile([P, FREE], F32, tag="ps", name=f"pg{g}_{i}")
                    for i in range(width * TH)
                ]
                base_loop(g, width, KQ * ooff * P, 2 * ooff * P, pg)

                j0 = 0
                if g == 0:
                    # evict the A tile -> hmask (scaled, masked); no delta
                    for th in range(TH):
                        tsl = slice(th * FREE, (th + 1) * FREE)
                        nc.vector.tensor_mul(
                            hmask[:, tsl], pg[th][:], maskT_sb[:, tsl])
                    j0 = 1
                # per-j: delta matmul, then evict+bias, then out DMA
                for j in range(j0, width):
                    om = ooff + j - 1  # real W o128-tile index
                    for th in range(TH):
                        tsl = slice(th * FREE, (th + 1) * FREE)
                        nc.tensor.matmul(
                            pg[j * TH + th][:],
                            bcat_sb[:, om * P:(om + 1) * P],
                            hmask[:, tsl],
                            start=False, stop=True,
                        )
                    ob = opool.tile([P, TOK], F32, tag="ob", name=f"ob_{om}")
                    for th in range(TH):
                        tsl = slice(th * FREE, (th + 1) * FREE)
                        nc.vector.tensor_scalar_add(
                            ob[:, tsl], pg[j * TH + th][:],
                            biasr_sb[:, om:om + 1],
                        )
                    if g == last_g:
                        # tail: low-latency HWDGE sync ring (idle by now)
                        nc.sync.dma_start(
                            out=outT[om * P:(om + 1) * P, :], in_=ob[:]
                        )
                    else:
                        nc.gpsimd.dma_start(
                            out=outT[om * P:(om + 1) * P, :], in_=ob[:]
                        )
                ooff += width

    nc.compile()
    return nc


def _prep_inputs(x, weight, bias, lora_a, lora_b, scalings, lora_mapping):
    x = np.ascontiguousarray(x, dtype=np.float32)
    weight = np.ascontiguousarray(weight, dtype=np.float32)
    bias = np.ascontiguousarray(bias, dtype=np.float32)
    lora_a = np.ascontiguousarray(lora_a, dtype=np.float32)
    lora_b = np.ascontiguousarray(lora_b, dtype=np.float32)
    scalings = np.ascontiguousarray(scalings, dtype=np.float32)
    lora_mapping = np.asarray(lora_mapping)

    xT = np.ascontiguousarray(x.T.astype(ml_dtypes.bfloat16))        # [D_IN, N_TOK]
    aT = lora_a.transpose(2, 0, 1).reshape(D_IN, AR)                 # [D_IN,(a r)]
    awT = np.concatenate([aT, weight.T], axis=1)                     # [D_IN, NO*P]
    # quad-major [A|W] with per-group contiguous (kk, cols) blocks
    w4 = awT.reshape(NQ, KQ, P, NO * P)                              # [kq,kk,p,o]
    blocks = []
    o0 = 0
    for wdt in GROUPS:
        blk = w4[:, :, :, o0:o0 + wdt * P]                           # [kq,kk,p,w]
        blocks.append(blk.transpose(0, 2, 1, 3).reshape(NQ, P, KQ * wdt * P))
        o0 += wdt * P
    wTr = np.ascontiguousarray(
        np.concatenate(blocks, axis=2).astype(ml_dtypes.bfloat16))

    # trailing k-chunks of [A|W] in fp8, pair-major for DoubleRow:
    # w8r[q, p, (per-group [i(2), width*P] blocks)]
    a8 = (awT[KC8 * P:, :] * F8SCALE).astype(
        ml_dtypes.float8_e4m3)                            # [2*F8PAIRS*P, NO*P]
    a8r = a8.reshape(F8PAIRS, 2, P, NO * P)
    blocks8 = []
    o0 = 0
    for wdt in GROUPS:
        blk8 = a8r[:, :, :, o0:o0 + wdt * P]              # [q,i,p,w]
        blocks8.append(blk8.transpose(0, 2, 1, 3).reshape(F8PAIRS, P, 2 * wdt * P))
        o0 += wdt * P
    w8r = np.ascontiguousarray(np.concatenate(blocks8, axis=2))

    bcat = np.ascontiguousarray(
        lora_b.transpose(0, 2, 1).reshape(AR, D_OUT)
        .astype(ml_dtypes.bfloat16))                                 # [(a r), D_OUT]
    # biasr[p, m] = bias[m*128 + p]
    biasr = np.ascontiguousarray(bias.reshape(D_OUT // P, P).T)      # [P, 32]
    # maskT[(a r), n] = scalings[a] * (lora_mapping[n] == a+1)
    ids = np.arange(1, N_ADAPTERS + 1, dtype=lora_mapping.dtype)
    onehot = (lora_mapping[None, :] == ids[:, None]).astype(np.float32)  # [A, N]
    maskT = (onehot * scalings[:, None]).repeat(RANK, axis=0)        # [(a r), N]
    maskT = np.ascontiguousarray(maskT)

    # fp8 copy of the trailing x^T k-chunks, pair-major: x8[q, p, i, n]
    x8full = np.ascontiguousarray(x.T[KC8 * P:, :] / F8SCALE).astype(
        ml_dtypes.float8_e4m3)                             # [2*F8PAIRS*P, N_TOK]
    x8full = x8full.reshape(F8PAIRS, 2, P, N_TOK).transpose(0, 2, 1, 3)

    in_maps = []
    for c in range(N_CORES):
        tsl = slice(c * TOK, (c + 1) * TOK)
        in_maps.append({
            "xT": np.ascontiguousarray(xT[:, tsl]),
            "wTr": wTr,
            "bcat": bcat,
            "maskT": np.ascontiguousarray(maskT[:, tsl]),
            "biasr": biasr,
            "x8": np.ascontiguousarray(x8full[:, :, :, tsl]),
            "w8r": w8r,
        })
    return in_maps


def run(inputs, trace=False):
    if "nc" not in _CACHE:
        _CACHE["nc"] = _build_nc()
    nc = _CACHE["nc"]
    in_maps = _prep_inputs(**inputs)
    res = run_bass_kernel_spmd(
        nc, in_maps, list(range(N_CORES)), trace=trace,
    )
    out = np.concatenate(
        [np.ascontiguousarray(r["outT"].T) for r in res.results], axis=0
    )
    return out, res


def kernel(**inputs) -> np.ndarray:
    out, _ = run(inputs, trace=False)
    return out


# revision 17
# speedup vs baseline: 1.0230x; 1.0230x over previous
"""LiteLinear (dense linear + per-token LoRA adapters) on 8 Trainium2 cores.

Sharding: data-parallel over tokens. Each core computes 1024 tokens:
  out = x @ W^T + bias + per-token LoRA delta.

Device kernel (per core), mixed bf16 / fp8 matmuls (fp32 PSUM accumulate):
  - Computes out^T [D_OUT x TOK]; host transposes back on assembly.
  - Stationary operand = weight sub-chunk [128d x 128o], moving = x^T
    [128d x 512tok]. x^T resident in SBUF (bf16, 2KB/partition/tile).
  - A_cat^T (the concatenated LoRA down-projections) is prepended to W^T
    as a 33rd output column tile, so h^T = A_cat @ x^T rides the same
    streamed matmul pipeline; its eviction is a DVE multiply with a
    host-built maskT (folds scalings + one-hot) producing hmask^T (bf16).
  - The combined [A|W]^T stream is re-laid-out on the host in quad-major
    form: one dma_start per 4 contraction chunks, 2KB contiguous lines.
  - The trailing 2*F8PAIRS k-chunks of every accumulation run as fp8-e4m3
    DoubleRow matmuls (2 k-chunks per instruction, native scale, same PSUM
    bank) -- 2x PE rate on that slice of the contraction for a measured
    rel-err of 1.56e-2 against the 2e-2 gate (error scales as
    sqrt(F8PAIRS) * 1.05e-2; 4 pairs would hit 90% of the gate).
  - Group 0 is width 4 (A + 3 W-tiles) so its matmul pace (8/k-chunk)
    stays behind the x^T DMA delivery pace; later groups are width 2 so
    only 4 PSUM banks are held per group and the next group's
    accumulation overlaps this group's delta/eviction.
  - 16 junk warmup matmuls run during the DMA preamble so the PE's HAM
    clock gate reaches 8/8 (2.4 GHz) before the first real matmul.
  - Per-token LoRA delta enters each out-tile as one extra accumulating
    matmul (lhsT=B_cat chunk bf16, rhs=hmask^T); bias folded into
    PSUM->SBUF eviction via per-partition tensor_scalar_add.
  - DMA rings: W quads on sync (HWDGE), x^T on scalar (HWDGE),
    consts + output tiles on gpsimd (SWDGE); the final o-tile's output
    rides sync to cut the kernel tail.
"""

import numpy as np

import sys

if "/opt/trn_rl_repo" not in sys.path:
    sys.path.insert(0, "/opt/trn_rl_repo")

import ml_dtypes

import concourse.bass as bass
import concourse.mybir as mybir
import concourse.tile as tile
from concourse import bacc
from concourse.bass_utils import run_bass_kernel_spmd

N_TOK = 8192
D_IN = 4096
D_OUT = 4096
N_ADAPTERS = 8
RANK = 16
AR = N_ADAPTERS * RANK  # 128
N_CORES = 8
TOK = N_TOK // N_CORES  # 1024 tokens per core

P = 128            # partitions
FREE = 512         # matmul moving free dim (== 1 PSUM bank of fp32)
KC = D_IN // P     # 32 contraction chunks
KQ = 4             # k-chunks per quad DMA
NQ = KC // KQ      # 8 quads
TH = TOK // FREE   # 2 token halves
NO = D_OUT // P + 1  # 33 o128-tiles incl. the A tile (index 0)
GROUPS = [4] + [2] * 14 + [1]  # o128-tiles per group (sum 33); g0 = [A,W0..W2]
NWARM = 16         # junk matmuls to lift the HAM clock gate during preamble
F8PAIRS = 4        # trailing k-chunk pairs done as fp8 DoubleRow (2 chunks/mm)
F8SCALE = 8.0      # power-of-2 split scale: W*s, x/s -- exact cancellation,
                   # rebalances both operands into e4m3 normal range
KC8 = KC - 2 * F8PAIRS  # k-chunks done in bf16 (28)

F32 = mybir.dt.float32
BF16 = mybir.dt.bfloat16
F8 = mybir.dt.float8e4

_CACHE = {}


def _build_nc():
    nc = bacc.Bacc(None, target_bir_lowering=False, debug=True)

    xT = nc.dram_tensor("xT", [D_IN, TOK], BF16, kind="ExternalInput")
    # quad-major [A|W]: [kq, p, (g kk cols_g)] with per-group contiguous blocks
    wTr = nc.dram_tensor("wTr", [NQ, P, KQ * NO * P], BF16,
                         kind="ExternalInput")
    bcat = nc.dram_tensor("bcat", [AR, D_OUT], BF16, kind="ExternalInput")
    maskT = nc.dram_tensor("maskT", [AR, TOK], F32, kind="ExternalInput")
    biasr = nc.dram_tensor("biasr", [P, D_OUT // P], F32, kind="ExternalInput")
    # trailing k-chunks in fp8, DoubleRow pair-major layouts
    x8 = nc.dram_tensor("x8", [F8PAIRS, P, 2, TOK], F8, kind="ExternalInput")
    w8r = nc.dram_tensor("w8r", [F8PAIRS, P, 2 * NO * P], F8,
                         kind="ExternalInput")
    outT = nc.dram_tensor("outT", [D_OUT, TOK], F32, kind="ExternalOutput")

    def w_src(kq, goff, coff, blk):
        """Slice of quad kq: per-(partition) rows, blk contiguous cols
        starting at goff+coff within the group-blocked column space."""
        return bass.AP(
            tensor=wTr[:].tensor,
            offset=kq * P * KQ * NO * P + goff + coff,
            ap=[[KQ * NO * P, P], [1, blk]],
        )

    def w8_src(q, goff8, blk):
        return bass.AP(
            tensor=w8r[:].tensor,
            offset=q * P * 2 * NO * P + goff8,
            ap=[[2 * NO * P, P], [1, blk]],
        )

    def x8_src(q):
        return bass.AP(
            tensor=x8[:].tensor,
            offset=q * P * 2 * TOK,
            ap=[[2 * TOK, P], [TOK, 2], [1, TOK]],
        )

    with tile.TileContext(nc) as tc:
        with (
            tc.tile_pool(name="xpool", bufs=1) as xpool,
            tc.tile_pool(name="const", bufs=1) as const,
            tc.tile_pool(name="wpool", bufs=3) as wpool,
            tc.tile_pool(name="opool", bufs=3) as opool,
            tc.tile_pool(name="psum", bufs=8, space="PSUM") as psum,
        ):
            hmask = const.tile([P, TOK], BF16, tag="hmask")
            biasr_sb = const.tile([P, D_OUT // P], F32, tag="biasr")
            maskT_sb = const.tile([P, TOK], F32, tag="maskT")
            bcat_sb = const.tile([P, D_OUT], BF16, tag="bcat")
            wjunk = const.tile([P, P], BF16, tag="wjunk")
            xjunk = const.tile([P, FREE], BF16, tag="xjunk")

            # PE warmup: junk matmuls with no DMA deps keep the PE busy
            # through the ~9us preamble so HAM unthrottles to 2.4 GHz.
            nc.vector.memset(wjunk[:], 0.0)
            nc.vector.memset(xjunk[:], 0.0)
            pwarm = [psum.tile([P, FREE], F32, tag="ps", name=f"pwarm{i}")
                     for i in range(2)]
            for i in range(NWARM):
                nc.tensor.matmul(pwarm[i % 2][:], wjunk[:], xjunk[:],
                                 start=True, stop=True)


            # consts ride the gpsimd ring (idle until output DMAs start)
            nc.gpsimd.dma_start(out=biasr_sb[:], in_=biasr[:, :])
            nc.gpsimd.dma_start(out=maskT_sb[:], in_=maskT[:, :])
            nc.gpsimd.dma_start(out=bcat_sb[:], in_=bcat[:, :])

            # x^T fully prefetched on the scalar ring (8 MB bf16)
            xt = []
            for k in range(KC):
                t = xpool.tile([P, TOK], BF16, tag=f"xt{k}", name=f"xt{k}")
                nc.scalar.dma_start(out=t[:], in_=xT[k * P:(k + 1) * P, :])
                xt.append(t)

            x8t = []
            for q in range(F8PAIRS):
                t = xpool.tile([P, 2, TOK], F8, tag=f"x8_{q}", name=f"x8_{q}")
                nc.scalar.dma_start(out=t[:], in_=x8_src(q))
                x8t.append(t)

            def moving(k, th):
                tsl = slice(th * FREE, (th + 1) * FREE)
                return xt[k][:, tsl]

            def base_loop(g, width, goff, goff8, pg):
                """28 bf16 k-chunks + 2 fp8-DR pairs for one o-group."""
                wt = None
                wt0 = [None] * KQ
                w8t = [None] * F8PAIRS
                for k in range(KC8):
                    if g == 0 and k < KQ:
                        # first quad split per-k for a faster first matmul
                        if k == 0:
                            for kk in range(KQ):
                                t = wpool.tile([P, width * P], BF16,
                                               tag="wt0", name=f"wt00_{kk}", bufs=4)
                                nc.sync.dma_start(
                                    out=t[:],
                                    in_=w_src(0, goff, kk * width * P,
                                              width * P))
                                wt0[kk] = t
                    elif k % KQ == 0:
                        wt = wpool.tile([P, KQ * width * P], BF16,
                                        tag=f"wt{width}", name=f"wt{g}_{k}")
                        nc.sync.dma_start(
                            out=wt[:],
                            in_=w_src(k // KQ, goff, 0, KQ * width * P))
                    if k == KC8 - KQ:
                        # fp8 stationaries ride sync behind the last quad
                        for q in range(F8PAIRS):
                            t8 = wpool.tile([P, 2, width * P], F8,
                                            tag=f"w8_{width}",
                                            name=f"w8_{g}_{q}",
                                            bufs=F8PAIRS + 1)
                            nc.sync.dma_start(
                                out=t8[:], in_=w8_src(q, goff8, 2 * width * P))
                            w8t[q] = t8
                    kk = k % KQ
                    for j in range(width):
                        if g == 0 and k < KQ:
                            lhsT = wt0[kk][:, j * P:(j + 1) * P]
                        else:
                            lhsT = wt[:, (kk * width + j) * P:
                                      (kk * width + j + 1) * P]
                        for th in range(TH):
                            nc.tensor.matmul(
                                pg[j * TH + th][:],
                                lhsT,
                                moving(k, th),
                                start=(k == 0),
                                stop=False,
                            )
                # trailing contraction: fp8 DoubleRow, 2 k-chunks per matmul
                for q in range(F8PAIRS):
                    for j in range(width):
                        lhsT8 = w8t[q][:, :, j * P:(j + 1) * P]
                        for th in range(TH):
                            tsl = slice(th * FREE, (th + 1) * FREE)
                            nc.tensor.matmul(
                                pg[j * TH + th][:],
                                lhsT8,
                                x8t[q][:, :, tsl],
                                start=False,
                                stop=(q == F8PAIRS - 1 and g == 0 and j == 0),
                                perf_mode=mybir.MatmulPerfMode.DoubleRow,
                            )

            ooff = 0  # in o128-tiles over the combined [A|W] column space
            last_g = len(GROUPS) - 1
            for g, width in enumerate(GROUPS):
                pg = [
                    psum.tile([P, FREE], F32, tag="ps", name=f"pg{g}_{i}")
                    for i in range(width * TH)
                ]
                base_loop(g, width, KQ * ooff * P, 2 * ooff * P, pg)

                j0 = 0
                if g == 0:
                    # evict the A tile -> hmask (scaled, masked); no delta
                    for th in range(TH):
                        tsl = slice(th * FREE, (th + 1) * FREE)
                        nc.vector.tensor_mul(
                            hmask[:, tsl], pg[th][:], maskT_sb[:, tsl])
                    j0 = 1
                # per-j: delta matmul, then evict+bias, then out DMA
                for j in range(j0, width):
                    om = ooff + j - 1  # real W o128-tile index
                    for th in range(TH):
                        tsl = slice(th * FREE, (th + 1) * FREE)
                        nc.tensor.matmul(
                            pg[j * TH + th][:],
                            bcat_sb[:, om * P:(om + 1) * P],
                            hmask[:, tsl],
                            start=False, stop=True,
                        )
                    ob = opool.tile([P, TOK], F32, tag="ob", name=f"ob_{om}")
                    for th in range(TH):
                        tsl = slice(th * FREE, (th + 1) * FREE)
                        nc.vector.tensor_scalar_add(
                            ob[:, tsl], pg[j * TH + th][:],
                            biasr_sb[:, om:om + 1],
                        )
                    if g == last_g:
                        # tail: low-latency HWDGE sync ring (idle by now)
                        nc.sync.dma_start(
                            out=outT[om * P:(om + 1) * P, :], in_=ob[:]
                        )
                    else:
                        nc.gpsimd.dma_start(
                            out=outT[om * P:(om + 1) * P, :], in_=ob[:]
                        )
                ooff += width

    nc.compile()
    return nc


def _prep_inputs(x, weight, bias, lora_a, lora_b, scalings, lora_mapping):
    x = np.ascontiguousarray(x, dtype=np.float32)
    weight = np.ascontiguousarray(weight, dtype=np.float32)
    bias = np.ascontiguousarray(bias, dtype=np.float32)
    lora_a = np.ascontiguousarray(lora_a, dtype=np.float32)
    lora_b = np.ascontiguousarray(lora_b, dtype=np.float32)
    scalings = np.ascontiguousarray(scalings, dtype=np.float32)
    lora_mapping = np.asarray(lora_mapping)

    xT = np.ascontiguousarray(x.T.astype(ml_dtypes.bfloat16))        # [D_IN, N_TOK]
    aT = lora_a.transpose(2, 0, 1).reshape(D_IN, AR)                 # [D_IN,(a r)]
    awT = np.concatenate([aT, weight.T], axis=1)                     # [D_IN, NO*P]
    # quad-major [A|W] with per-group contiguous (kk, cols) blocks
    w4 = awT.reshape(NQ, KQ, P, NO * P)                              # [kq,kk,p,o]
    blocks = []
    o0 = 0
    for wdt in GROUPS:
        blk = w4[:, :, :, o0:o0 + wdt * P]                           # [kq,kk,p,w]
        blocks.append(blk.transpose(0, 2, 1, 3).reshape(NQ, P, KQ * wdt * P))
        o0 += wdt * P
    wTr = np.ascontiguousarray(
        np.concatenate(blocks, axis=2).astype(ml_dtypes.bfloat16))

    # trailing k-chunks of [A|W] in fp8, pair-major for DoubleRow:
    # w8r[q, p, (per-group [i(2), width*P] blocks)]
    a8 = (awT[KC8 * P:, :] * F8SCALE).astype(
        ml_dtypes.float8_e4m3)                            # [2*F8PAIRS*P, NO*P]
    a8r = a8.reshape(F8PAIRS, 2, P, NO * P)
    blocks8 = []
    o0 = 0
    for wdt in GROUPS:
        blk8 = a8r[:, :, :, o0:o0 + wdt * P]              # [q,i,p,w]
        blocks8.append(blk8.transpose(0, 2, 1, 3).reshape(F8PAIRS, P, 2 * wdt * P))
        o0 += wdt * P
    w8r = np.ascontiguousarray(np.concatenate(blocks8, axis=2))

    bcat = np.ascontiguousarray(
        lora_b.transpose(0, 2, 1).reshape(AR, D_OUT)
        .astype(ml_dtypes.bfloat16))                                 # [(a r), D_OUT]
    # biasr[p, m] = bias[m*128 + p]
    biasr = np.ascontiguousarray(bias.reshape(D_OUT // P, P).T)      # [P, 32]
    # maskT[(a r), n] = scalings[a] * (lora_mapping[n] == a+1)
    ids = np.arange(1, N_ADAPTERS + 1, dtype=lora_mapping.dtype)
    onehot = (lora_mapping[None, :] == ids[:, None]).astype(np.float32)  # [A, N]
    maskT = (onehot * scalings[:, None]).repeat(RANK, axis=0)        # [(a r), N]
    maskT = np.ascontiguousarray(maskT)

    # fp8 copy of the trailing x^T k-chunks, pair-major: x8[q, p, i, n]
    x8full = np.ascontiguousarray(x.T[KC8 * P:, :] / F8SCALE).astype(
        ml_dtypes.float8_e4m3)                             # [2*F8PAIRS*P, N_TOK]
    x8full = x8full.reshape(F8PAIRS, 2, P, N_TOK).transpose(0, 2, 1, 3)

    in_maps = []
    for c in range(N_CORES):
        tsl = slice(c * TOK, (c + 1) * TOK)
        in_maps.append({
            "xT": np.ascontiguousarray(xT[:, tsl]),
            "wTr": wTr,
            "bcat": bcat,
            "maskT": np.ascontiguousarray(maskT[:, tsl]),
            "biasr": biasr,
            "x8": np.ascontiguousarray(x8full[:, :, :, tsl]),
            "w8r": w8r,
        })
    return in_maps


def run(inputs, trace=False):
    if "nc" not in _CACHE:
        _CACHE["nc"] = _build_nc()
    nc = _CACHE["nc"]
    in_maps = _prep_inputs(**inputs)
    res = run_bass_kernel_spmd(
        nc, in_maps, list(range(N_CORES)), trace=trace,
    )
    out = np.concatenate(
        [np.ascontiguousarray(r["outT"].T) for r in res.results], axis=0
    )
    return out, res


def kernel(**inputs) -> np.ndarray:
    out, _ = run(inputs, trace=False)
    return out
